# revision 1
# baseline (speedup 1.0000x reference)
"""Trainium2 Bass kernel for nn_JslBERT (embedding lookup + 4-layer BERT encoder).

Sharding: 8 cores = 4 batch x 2 head-groups. Core c handles batch b=c//2 and
heads [6g, 6g+6) with g=c%2. Per layer, the attention-output partials are
pairwise AllReduced; LN+FFN run redundantly on both cores of a pair.

All matmuls run in float32r (TF32-like, 1 cycle/row on the PE for N>=256,
rel err ~1.6e-4 per matmul). PSUM accumulation, softmax and layernorm are fp32.
"""
import numpy as np

import concourse.bass as bass
import concourse.bacc as bacc
import concourse.tile as tile
import concourse.bass_utils as bass_utils
from concourse import mybir
from concourse.masks import make_identity

# Model dims (hardcoded per problem spec)
B, S, L, D, H, V, PMAX = 4, 512, 4, 768, 12, 32000, 512
EPS = 1e-3
NCORES = 8
HPC = H // 2          # heads per core
KH = D                # head dim (768)
HK = HPC * KH         # 4608 flattened head dims per core
SCALE = 1.0 / float(np.sqrt(D))

F32 = mybir.dt.float32
F32R = mybir.dt.float32r
I32 = mybir.dt.int32

# t tiles (S=512 -> 4), d chunks (D=768 -> 6 of 128), output free-dim chunks
TT = S // 128         # 4
DC = D // 128         # 6
NCH = [(0, 512), (512, 256)]  # free-dim chunks for width-768 outputs


def build_nc(n_layers=L, flags=None):
    """Build the Bass graph. flags: dict of which optional inputs exist."""
    flags = flags or {}
    nc = bacc.Bacc("TRN2", target_bir_lowering=False, debug=False,
                   num_devices=NCORES)

    xids_d = nc.dram_tensor("xids", [3, S], I32, kind="ExternalInput").ap()
    tokw_d = nc.dram_tensor("tok_w", [V, D], F32, kind="ExternalInput").ap()
    posw_d = nc.dram_tensor("pos_w", [PMAX, D], F32, kind="ExternalInput").ap()
    segw_d = nc.dram_tensor("seg_w", [2, D], F32, kind="ExternalInput").ap()
    wq_d = nc.dram_tensor("wq", [n_layers, D, HK], F32R, kind="ExternalInput").ap()
    wk_d = nc.dram_tensor("wk", [n_layers, D, HK], F32R, kind="ExternalInput").ap()
    wv_d = nc.dram_tensor("wv", [n_layers, D, HK], F32R, kind="ExternalInput").ap()
    wo_d = nc.dram_tensor("wo", [n_layers, HK, D], F32R, kind="ExternalInput").ap()
    ff_d = nc.dram_tensor("ff", [n_layers, D, D], F32R, kind="ExternalInput").ap()
    out_d = nc.dram_tensor("out", [S, D], F32, kind="ExternalOutput").ap()

    # optional general-path inputs (skipped when zero / identity)
    opt = {}
    if flags.get("emb_bias"):
        opt["emb_bias"] = nc.dram_tensor("emb_bias", [D], F32, kind="ExternalInput").ap()
    if flags.get("bqkv"):
        opt["bqkv"] = nc.dram_tensor("bqkv", [3, n_layers, HK], F32, kind="ExternalInput").ap()
    if flags.get("bo"):
        opt["bo"] = nc.dram_tensor("bo", [n_layers, D], F32, kind="ExternalInput").ap()
    if flags.get("ffb"):
        opt["ffb"] = nc.dram_tensor("ffb", [n_layers, D], F32, kind="ExternalInput").ap()
    for nm in ("ln1", "ln2"):
        if flags.get(nm):
            opt[nm + "_g"] = nc.dram_tensor(nm + "_g", [n_layers, D], F32, kind="ExternalInput").ap()
            opt[nm + "_b"] = nc.dram_tensor(nm + "_b", [n_layers, D], F32, kind="ExternalInput").ap()
    if flags.get("mask"):
        opt["maskneg"] = nc.dram_tensor("maskneg", [S], F32, kind="ExternalInput").ap()

    with tile.TileContext(nc) as tc:
        import contextlib
        with contextlib.ExitStack() as ctx:
            _build_body(ctx, tc, n_layers, flags, xids_d, tokw_d, posw_d, segw_d,
                        wq_d, wk_d, wv_d, wo_d, ff_d, out_d, opt)
    nc.compile()
    return nc


def _build_body(ctx, tc, n_layers, flags, xids_d, tokw_d, posw_d, segw_d,
                wq_d, wk_d, wv_d, wo_d, ff_d, out_d, opt):
    nc = tc.nc

    const = ctx.enter_context(tc.tile_pool(name="const", bufs=1))
    w_pool = ctx.enter_context(tc.tile_pool(name="wp", bufs=26))
    rT_pool = ctx.enter_context(tc.tile_pool(name="rT", bufs=7))
    xtd_pool = ctx.enter_context(tc.tile_pool(name="xtd", bufs=9))
    qk_pool = ctx.enter_context(tc.tile_pool(name="qk", bufs=13))
    v_pool = ctx.enter_context(tc.tile_pool(name="vp", bufs=5))
    p_pool = ctx.enter_context(tc.tile_pool(name="pp", bufs=4))
    pt_pool = ctx.enter_context(tc.tile_pool(name="pt", bufs=4))
    ct_pool = ctx.enter_context(tc.tile_pool(name="ct", bufs=7))
    sm_pool = ctx.enter_context(tc.tile_pool(name="sm", bufs=24))
    ps_mm = ctx.enter_context(tc.tile_pool(name="psmm", bufs=4, space="PSUM"))
    ps_tp = ctx.enter_context(tc.tile_pool(name="pstp", bufs=3, space="PSUM"))
    dram = ctx.enter_context(tc.tile_pool(name="dram", bufs=1, space="DRAM"))

    ident = const.tile([128, 128], F32)
    make_identity(nc, ident[:])
    eps_t = const.tile([128, 1], F32)
    nc.vector.memset(eps_t[:], EPS)

    def mm_tile():
        return ps_mm.tile([128, 512], F32, tag="mm", name="mmps")

    def tp_tile():
        return ps_tp.tile([128, 128], F32, tag="tp", name="tpps")

    # ---- transpose [t,d]-tiles -> [d,t] fp32r tiles --------------------
    def transpose_to_dT(src_tiles, bias_ap=None):
        """src_tiles: TT tiles [128, D] fp32. Returns DC tiles [128, S] f32r."""
        dst = [rT_pool.tile([128, S], F32R, tag="rT", name=f"dT{dc}") for dc in range(DC)]
        for dc in range(DC):
            for tm in range(TT):
                pt = tp_tile()
                nc.tensor.transpose(pt[:], src_tiles[tm][:, dc * 128:(dc + 1) * 128], ident[:])
                dstsl = dst[dc][:, tm * 128:(tm + 1) * 128]
                if bias_ap is not None:
                    nc.vector.tensor_scalar_add(dstsl, pt[:], bias_ap[dc])
                else:
                    nc.any.tensor_copy(out=dstsl, in_=pt[:])
        return dst

    # ---- embeddings ----------------------------------------------------
    idx = const.tile([128, 3, TT], I32)
    nc.sync.dma_start(idx[:], xids_d.rearrange("k (j p) -> p k j", p=128))

    emb_bias_ap = None
    if "emb_bias" in opt:
        eb = const.tile([128, DC], F32)
        nc.sync.dma_start(eb[:], opt["emb_bias"].rearrange("(c p) -> p c", p=128))
        emb_bias_ap = [eb[:, c:c + 1] for c in range(DC)]

    x_tiles = []
    for tm in range(TT):
        xt = xtd_pool.tile([128, D], F32, tag="xtd")
        tmp = xtd_pool.tile([128, D], F32, tag="xtd")
        nc.gpsimd.indirect_dma_start(
            out=xt[:], out_offset=None, in_=tokw_d[:],
            in_offset=bass.IndirectOffsetOnAxis(ap=idx[:, 0, tm:tm + 1], axis=0))
        nc.gpsimd.indirect_dma_start(
            out=tmp[:], out_offset=None, in_=posw_d[:],
            in_offset=bass.IndirectOffsetOnAxis(ap=idx[:, 1, tm:tm + 1], axis=0))
        nc.vector.tensor_add(xt[:], xt[:], tmp[:])
        tmp2 = xtd_pool.tile([128, D], F32, tag="xtd")
        nc.gpsimd.indirect_dma_start(
            out=tmp2[:], out_offset=None, in_=segw_d[:],
            in_offset=bass.IndirectOffsetOnAxis(ap=idx[:, 2, tm:tm + 1], axis=0))
        nc.vector.tensor_add(xt[:], xt[:], tmp2[:])
        x_tiles.append(xt)

    resT = transpose_to_dT(x_tiles, emb_bias_ap)

    mask_ap = None
    if "maskneg" in opt:
        mk = const.tile([128, S], F32)
        nc.sync.dma_start(mk[:], opt["maskneg"].partition_broadcast(128))
        mask_ap = mk

    # ---- layers --------------------------------------------------------
    arin = dram.tile([S, D], F32)
    arout = dram.tile([S, D], F32)

    for li in range(n_layers):
        acc = [xtd_pool.tile([128, D], F32, tag="xtd", name=f"acc{tm}") for tm in range(TT)]

        bq_ap = bk_ap = bv_ap = None
        if "bqkv" in opt:
            bq_sb = const.tile([128, 3, HK // 128], F32, tag=f"bqkv{li}")
            nc.sync.dma_start(bq_sb[:], opt["bqkv"][:, li, :].rearrange("k (c p) -> p k c", p=128))

        for h in range(HPC):
            # -- load this head's weights (DMA, fp32r)
            wq_sb, wk_sb, wv_sb = [], [], []
            for (wlist, wd) in ((wq_sb, wq_d), (wk_sb, wk_d), (wv_sb, wv_d)):
                for dc in range(DC):
                    wt = w_pool.tile([128, KH], F32R, tag="w")
                    nc.sync.dma_start(wt[:], wd[li, dc * 128:(dc + 1) * 128, h * KH:(h + 1) * KH])
                    wlist.append(wt)

            # -- QT, KT: [k, t] accumulation over d
            qt_sb, kt_sb = [], []
            for (dst, w_sb, kind) in ((qt_sb, wq_sb, 0), (kt_sb, wk_sb, 1)):
                for m in range(DC):
                    pm = mm_tile()
                    for dc in range(DC):
                        nc.tensor.matmul(pm[:], w_sb[dc][:, m * 128:(m + 1) * 128], resT[dc][:],
                                         start=(dc == 0), stop=(dc == DC - 1))
                    ot = qk_pool.tile([128, S], F32R, tag="qk")
                    if "bqkv" in opt:
                        nc.vector.tensor_scalar_add(ot[:], pm[:], bq_sb[:, kind, (h * KH) // 128 + m:(h * KH) // 128 + m + 1])
                    else:
                        nc.any.tensor_copy(out=ot[:], in_=pm[:])
                    dst.append(ot)

            # -- V: [s, k] accumulation over d
            v_sb = []
            for sm in range(TT):
                vt = v_pool.tile([128, KH], F32R, tag="v")
                for (n0, nw) in NCH:
                    pm = mm_tile()
                    for dc in range(DC):
                        nc.tensor.matmul(pm[:, :nw], resT[dc][:, sm * 128:(sm + 1) * 128],
                                         wv_sb[dc][:, n0:n0 + nw],
                                         start=(dc == 0), stop=(dc == DC - 1))
                    # bias bv over free dim: handled via rank-1 matmul in general
                    # case (omitted: zero in this problem)
                    nc.any.tensor_copy(out=vt[:, n0:n0 + nw], in_=pm[:, :nw])
                v_sb.append(vt)

            # -- scores + softmax (unstable: |scores| < ~1 by construction)
            p_tiles = []
            for tm in range(TT):
                pm = mm_tile()
                for kc in range(DC):
                    nc.tensor.matmul(pm[:], qt_sb[kc][:, tm * 128:(tm + 1) * 128], kt_sb[kc][:],
                                     start=(kc == 0), stop=(kc == DC - 1))
                pe = p_pool.tile([128, S], F32, tag="p")
                sums = sm_pool.tile([128, 1], F32, tag="sums")
                if mask_ap is not None:
                    nc.vector.tensor_add(pm[:], pm[:], mask_ap[:])
                nc.scalar.activation(out=pe[:], in_=pm[:], func=mybir.ActivationFunctionType.Exp,
                                     scale=SCALE, accum_out=sums[:])
                rec = sm_pool.tile([128, 1], F32, tag="rec")
                nc.vector.reciprocal(rec[:], sums[:])
                nc.vector.tensor_scalar_mul(pe[:], pe[:], rec[:])
                p_tiles.append(pe)

            # -- transpose P -> PT [s, t]
            pt_sb = [pt_pool.tile([128, S], F32R, tag="pt", name=f"ptsb{sc}") for sc in range(TT)]
            for tm in range(TT):
                for sc in range(TT):
                    pt = tp_tile()
                    nc.tensor.transpose(pt[:], p_tiles[tm][:, sc * 128:(sc + 1) * 128], ident[:])
                    nc.any.tensor_copy(out=pt_sb[sc][:, tm * 128:(tm + 1) * 128], in_=pt[:])

            # -- ctxT [k, t] = V.T @ PT
            ct_sb = []
            for km in range(DC):
                pm = mm_tile()
                for sc in range(TT):
                    nc.tensor.matmul(pm[:], v_sb[sc][:, km * 128:(km + 1) * 128], pt_sb[sc][:],
                                     start=(sc == 0), stop=(sc == TT - 1))
                ot = ct_pool.tile([128, S], F32R, tag="ct")
                nc.any.tensor_copy(out=ot[:], in_=pm[:])
                ct_sb.append(ot)

            # -- wo for this head
            wo_sb = []
            for kc in range(DC):
                wt = w_pool.tile([128, D], F32R, tag="w")
                nc.sync.dma_start(wt[:], wo_d[li, h * KH + kc * 128: h * KH + (kc + 1) * 128, :])
                wo_sb.append(wt)

            # -- out partial [t, d] += ctxT.T @ wo
            for tm in range(TT):
                for (n0, nw) in NCH:
                    pm = mm_tile()
                    for kc in range(DC):
                        nc.tensor.matmul(pm[:, :nw], ct_sb[kc][:, tm * 128:(tm + 1) * 128],
                                         wo_sb[kc][:, n0:n0 + nw],
                                         start=(kc == 0), stop=(kc == DC - 1))
                    if h == 0:
                        nc.any.tensor_copy(out=acc[tm][:, n0:n0 + nw], in_=pm[:, :nw])
                    else:
                        nc.vector.tensor_add(acc[tm][:, n0:n0 + nw], acc[tm][:, n0:n0 + nw], pm[:, :nw])

        # ---- pairwise AllReduce of out partials ----
        for tm in range(TT):
            nc.sync.dma_start(arin[tm * 128:(tm + 1) * 128, :], acc[tm][:])
        nc.gpsimd.collective_compute(
            "AllReduce", mybir.AluOpType.add,
            replica_groups=[[0, 1], [2, 3], [4, 5], [6, 7]],
            ins=[arin.opt()], outs=[arout.opt()])
        xcur = [xtd_pool.tile([128, D], F32, tag="xtd", name=f"xcur{tm}") for tm in range(TT)]
        for tm in range(TT):
            nc.sync.dma_start(xcur[tm][:], arout[tm * 128:(tm + 1) * 128, :])

        # ---- LN1 (+bo would fold here; zero in this problem) ----
        _layernorm(nc, sm_pool, const, xcur, eps_t,
                   opt.get("ln1_g"), opt.get("ln1_b"), li)

        # ---- transpose ln1 -> [d, t] for FFN ----
        lnT = transpose_to_dT(xcur)

        # ---- FFN: mid[t, d'] = ln1 @ F ----
        ff_sb = []
        for dc in range(DC):
            wt = w_pool.tile([128, D], F32R, tag="w")
            nc.sync.dma_start(wt[:], ff_d[li, dc * 128:(dc + 1) * 128, :])
            ff_sb.append(wt)
        xmid = [xtd_pool.tile([128, D], F32, tag="xtd", name=f"xmid{tm}") for tm in range(TT)]
        for tm in range(TT):
            for (n0, nw) in NCH:
                pm = mm_tile()
                for dc in range(DC):
                    nc.tensor.matmul(pm[:, :nw], lnT[dc][:, tm * 128:(tm + 1) * 128],
                                     ff_sb[dc][:, n0:n0 + nw],
                                     start=(dc == 0), stop=(dc == DC - 1))
                nc.any.tensor_copy(out=xmid[tm][:, n0:n0 + nw], in_=pm[:, :nw])

        # ---- LN2 ----
        _layernorm(nc, sm_pool, const, xmid, eps_t,
                   opt.get("ln2_g"), opt.get("ln2_b"), li)

        if li < n_layers - 1:
            resT = transpose_to_dT(xmid)
        else:
            for tm in range(TT):
                nc.sync.dma_start(out_d[tm * 128:(tm + 1) * 128, :], xmid[tm][:])


def _layernorm(nc, sm_pool, const, tiles, eps_t, g_d, b_d, li):
    """In-place layernorm over free dim (D=768) of TT tiles [128, D] fp32."""
    gb = None
    if g_d is not None:
        gb = const.tile([128, 2, D], F32, tag=f"lngb{li}{id(g_d) % 97}")
        nc.sync.dma_start(gb[:, 0, :], g_d[li].partition_broadcast(128))
        nc.sync.dma_start(gb[:, 1, :], b_d[li].partition_broadcast(128))
    for tm in range(len(tiles)):
        x = tiles[tm]
        stats = sm_pool.tile([128, 3, 6], F32, tag="bnst")
        mv = sm_pool.tile([128, 2], F32, tag="bnmv")
        xg = x[:].rearrange("p (a c) -> p a c", a=3)
        for a in range(3):
            nc.vector.bn_stats(out=stats[:, a, :], in_=xg[:, a, :])
        nc.vector.bn_aggr(out=mv[:], in_=stats[:])
        rstd = sm_pool.tile([128, 1], F32, tag="rstd")
        nc.scalar.activation(out=rstd[:], in_=mv[:, 1:2],
                             func=mybir.ActivationFunctionType.Sqrt,
                             bias=eps_t[:], scale=1.0)
        nc.vector.reciprocal(rstd[:], rstd[:])
        nc.vector.tensor_scalar(out=x[:], in0=x[:], scalar1=mv[:, 0:1], scalar2=rstd[:],
                                op0=mybir.AluOpType.subtract, op1=mybir.AluOpType.mult)
        if gb is not None:
            nc.vector.tensor_mul(x[:], x[:], gb[:, 0, :])
            nc.vector.tensor_add(x[:], x[:], gb[:, 1, :])


# ------------------------------------------------------------------------
# host side
# ------------------------------------------------------------------------
_CACHED = {}


def _get_nc(n_layers, flag_key, flags):
    key = (n_layers, flag_key)
    if key not in _CACHED:
        _CACHED[key] = build_nc(n_layers, flags)
    return _CACHED[key]


def kernel(X, tok_w, tok_b, pos_w, pos_b, seg_w, seg_b,
           Wq, bq, Wk, bk, Wv, bv, Wo, bo,
           ln1_g, ln1_b, ffp_w, ffp_b, ln2_g, ln2_b, n_layers=L):
    f32 = np.float32
    X = np.asarray(X, dtype=np.int32)
    tok_w = np.asarray(tok_w, f32); pos_w = np.asarray(pos_w, f32); seg_w = np.asarray(seg_w, f32)
    Wq = np.asarray(Wq, f32); Wk = np.asarray(Wk, f32); Wv = np.asarray(Wv, f32)
    Wo = np.asarray(Wo, f32); ffp_w = np.asarray(ffp_w, f32)
    bq = np.asarray(bq, f32); bk = np.asarray(bk, f32); bv = np.asarray(bv, f32)
    bo = np.asarray(bo, f32); ffp_b = np.asarray(ffp_b, f32)
    ln1_g = np.asarray(ln1_g, f32); ln1_b = np.asarray(ln1_b, f32)
    ln2_g = np.asarray(ln2_g, f32); ln2_b = np.asarray(ln2_b, f32)
    tok_b = np.asarray(tok_b, f32); pos_b = np.asarray(pos_b, f32); seg_b = np.asarray(seg_b, f32)

    emb_bias = tok_b + pos_b + seg_b
    flags = {
        "emb_bias": bool(np.any(emb_bias)),
        "bqkv": bool(np.any(bq) or np.any(bk) or np.any(bv)),
        "bo": bool(np.any(bo)),
        "ffb": bool(np.any(ffp_b)),
        "ln1": bool(np.any(ln1_g != 1) or np.any(ln1_b)),
        "ln2": bool(np.any(ln2_g != 1) or np.any(ln2_b)),
        "mask": bool(np.any(X[:, 0, :] == 0)),
    }
    assert not (flags["bo"] or flags["ffb"] or flags["bqkv"]), \
        "nonzero attention/ffn biases not implemented in this specialization"
    flag_key = tuple(sorted(flags.items()))
    nc = _get_nc(n_layers, flag_key, flags)

    in_maps = []
    for c in range(NCORES):
        b, g = c // 2, c % 2
        hsl = slice(g * HPC, (g + 1) * HPC)
        m = {
            "xids": np.ascontiguousarray(X[b]),
            "tok_w": tok_w, "pos_w": pos_w, "seg_w": seg_w,
            "wq": np.ascontiguousarray(Wq[:n_layers, :, hsl, :]).reshape(n_layers, D, HK),
            "wk": np.ascontiguousarray(Wk[:n_layers, :, hsl, :]).reshape(n_layers, D, HK),
            "wv": np.ascontiguousarray(Wv[:n_layers, :, hsl, :]).reshape(n_layers, D, HK),
            "wo": np.ascontiguousarray(Wo[:n_layers, hsl, :, :]).reshape(n_layers, HK, D),
            "ff": np.ascontiguousarray(ffp_w[:n_layers]),
        }
        if flags["emb_bias"]:
            m["emb_bias"] = emb_bias
        if flags["ln1"]:
            m["ln1_g"] = np.ascontiguousarray(ln1_g[:n_layers])
            m["ln1_b"] = np.ascontiguousarray(ln1_b[:n_layers])
        if flags["ln2"]:
            m["ln2_g"] = np.ascontiguousarray(ln2_g[:n_layers])
            m["ln2_b"] = np.ascontiguousarray(ln2_b[:n_layers])
        if flags["mask"]:
            m["maskneg"] = np.where(X[b, 0, :] == 0, -1e9, 0.0).astype(f32)
        in_maps.append(m)

    res = bass_utils.run_bass_kernel_spmd(nc, in_maps, core_ids=list(range(NCORES)))
    out = np.stack([res.results[2 * b]["out"] for b in range(B)])
    return out

